# revision 1
# baseline (speedup 1.0000x reference)
"""Trainium2 Bass kernel for nn_AttentionLayer (B=2, S=2048, HID=1024, 16 heads x 64).

Sharding: 8 cores = 2 batches x 4 head-groups (4 heads each). Each core computes
its batch's attention for its 4 heads and writes a disjoint [256, 2048] slice of
the output (transposed). No collectives.

Device math (per core, all layouts feature-major to match TensorE):
  Q.T = Wq_g.T @ X_f.T + bq_g      [256 j, 2048 f]   (f32r matmuls)
  K.T = Wk_g.T @ X_t.T + bk_g      [256 j, 2048 t]
  V'  = [X_t.T.T @ Wv_g + 1*bv_g | ones]  [2048 t, 4, 65] (bf16)
  S.T = K.T_h.T @ Q.T_h            [t, f] per head    (row-packed pairs, K=64)
  E   = exp(S.T / 8)               (bf16, ScalarE)
  ctx'.T = V'_h.T @ E              [65, f]; row 64 = softmax denominator
  out = ctx'.T[0:64] * (1/denom broadcast)            (already includes bv)
"""
import numpy as np

B, S, HID = 2, 2048, 1024
NUM_HEADS, HEAD_DIM = 16, 64
G = 4                 # head-groups (cores per batch)
HPC = 4               # heads per core
JW = HPC * HEAD_DIM   # 256 W columns per core
NC_CHUNKS = HID // 128  # 8 contraction chunks
NT = S // 128         # 16 t tiles
NFB = 2               # f blocks of 1024
SCALE = 1.0 / np.sqrt(float(HEAD_DIM))

_cached = None


def _build():
    import contextlib
    import concourse.bass as bass
    import concourse.tile as tile
    from concourse.tile_rust import add_dep_helper
    from concourse import bacc, mybir

    F32R = mybir.dt.float32r
    F32 = mybir.dt.float32
    BF16 = mybir.dt.bfloat16
    Act = mybir.ActivationFunctionType

    nc = bacc.Bacc("TRN2", target_bir_lowering=False, debug=False, num_devices=8)

    fromT = nc.dram_tensor("fromT", (HID, S), F32R, kind="ExternalInput").ap()
    toT = nc.dram_tensor("toT", (HID, S), F32R, kind="ExternalInput").ap()
    wq = nc.dram_tensor("wq", (HID, JW), F32R, kind="ExternalInput").ap()
    wk = nc.dram_tensor("wk", (HID, JW), F32R, kind="ExternalInput").ap()
    wv = nc.dram_tensor("wv", (HID, JW), F32R, kind="ExternalInput").ap()
    bq = nc.dram_tensor("bq", (JW, 1), F32, kind="ExternalInput").ap()
    bk = nc.dram_tensor("bk", (JW, 1), F32, kind="ExternalInput").ap()
    bv = nc.dram_tensor("bv", (1, JW), F32R, kind="ExternalInput").ap()
    out = nc.dram_tensor("out", (JW, S), F32, kind="ExternalOutput").ap()

    with tile.TileContext(nc) as tc:
        with contextlib.ExitStack() as es:
            persist = es.enter_context(tc.tile_pool(name="persist", bufs=1))
            psbig = es.enter_context(tc.tile_pool(name="psbig", bufs=2, space="PSUM"))
            pssm = es.enter_context(tc.tile_pool(name="pssm", bufs=4, space="PSUM"))

            # --- constants / biases
            bq_sb = []
            bk_sb = []
            for jt in range(2):
                tq = persist.tile([128, 1], F32, tag=f"bq{jt}")
                nc.sync.dma_start(tq[:], bq[128 * jt:128 * jt + 128, 0:1])
                bq_sb.append(tq)
                tk = persist.tile([128, 1], F32, tag=f"bk{jt}")
                nc.sync.dma_start(tk[:], bk[128 * jt:128 * jt + 128, 0:1])
                bk_sb.append(tk)
            bv_row = persist.tile([1, JW], F32R, tag="bvrow")
            nc.sync.dma_start(bv_row[:], bv[0:1, :])
            ones_f = persist.tile([1, 128], F32, tag="onesf")
            nc.vector.memset(ones_f[:], 1.0)
            ones_r = persist.tile([1, 128], F32R, tag="onesr")
            nc.vector.tensor_copy(ones_r[:], ones_f[:])
            ones_bf = persist.tile([1, 64], BF16, tag="onesbf")
            nc.vector.memset(ones_bf[:], 1.0)

            # --- persistent projection outputs
            qt = [persist.tile([128, S], F32R, tag=f"qt{jt}", name=f"qt{jt}") for jt in range(2)]
            kt = [persist.tile([128, S], F32R, tag=f"kt{jt}", name=f"kt{jt}") for jt in range(2)]
            vp = [persist.tile([128, HPC, 65], BF16, tag=f"vp{tt}", name=f"vp{tt}") for tt in range(NT)]

            # --- phase A: projections
            with contextlib.ExitStack() as esA:
                pA = esA.enter_context(tc.tile_pool(name="phaseA", bufs=1))
                wq_sb = pA.tile([128, NC_CHUNKS, JW], F32R, tag="wq")
                nc.sync.dma_start(wq_sb[:], wq.rearrange("(c p) j -> p c j", p=128))
                wk_sb = pA.tile([128, NC_CHUNKS, JW], F32R, tag="wk")
                nc.sync.dma_start(wk_sb[:], wk.rearrange("(c p) j -> p c j", p=128))
                wv_sb = pA.tile([128, NC_CHUNKS, JW], F32R, tag="wv")
                nc.sync.dma_start(wv_sb[:], wv.rearrange("(c p) j -> p c j", p=128))

                # stream fromT/toT in column-halves of 1024 (32KB/partition
                # tiles) through a shared 3-slot pool to stay under the
                # ~196KB/partition SBUF budget the runtime leaves us.
                SH = 1024
                for h in range(2):
                    fx = pA.tile([128, NC_CHUNKS, SH], F32R, tag="xh",
                                 name=f"fromTh{h}", bufs=3)
                    for c in range(NC_CHUNKS):
                        nc.sync.dma_start(
                            fx[:, c, :],
                            fromT[128 * c:128 * c + 128, SH * h:SH * h + SH])
                    for jt in range(2):
                        for f2 in range(2):
                            fc = 2 * h + f2
                            acc = psbig.tile([128, 512], mybir.dt.float32, tag="big")
                            for c in range(NC_CHUNKS):
                                nc.tensor.matmul(
                                    acc[:],
                                    wq_sb[:, c, 128 * jt:128 * jt + 128],
                                    fx[:, c, 512 * f2:512 * f2 + 512],
                                    start=(c == 0), stop=(c == NC_CHUNKS - 1))
                            nc.scalar.activation(
                                qt[jt][:, 512 * fc:512 * fc + 512], acc[:],
                                Act.Identity, bias=bq_sb[jt][:], scale=1.0)
                for h in range(2):
                    tx = pA.tile([128, NC_CHUNKS, SH], F32R, tag="xh",
                                 name=f"toTh{h}", bufs=3)
                    for c in range(NC_CHUNKS):
                        nc.sync.dma_start(
                            tx[:, c, :],
                            toT[128 * c:128 * c + 128, SH * h:SH * h + SH])
                    for jt in range(2):
                        for f2 in range(2):
                            fc = 2 * h + f2
                            acc = psbig.tile([128, 512], mybir.dt.float32, tag="big")
                            for c in range(NC_CHUNKS):
                                nc.tensor.matmul(
                                    acc[:],
                                    wk_sb[:, c, 128 * jt:128 * jt + 128],
                                    tx[:, c, 512 * f2:512 * f2 + 512],
                                    start=(c == 0), stop=(c == NC_CHUNKS - 1))
                            nc.scalar.activation(
                                kt[jt][:, 512 * fc:512 * fc + 512], acc[:],
                                Act.Identity, bias=bk_sb[jt][:], scale=1.0)
                    # V tiles for this half: t tiles 8h .. 8h+7
                    for t2 in range(NT // 2):
                        tt = (NT // 2) * h + t2
                        accv = pssm.tile([128, 512], mybir.dt.float32, tag="sm")
                        nc.tensor.matmul(accv[:, 0:JW], ones_r[0:1, :], bv_row[0:1, :],
                                         start=True, stop=False)
                        for c in range(NC_CHUNKS):
                            nc.tensor.matmul(
                                accv[:, 0:JW],
                                tx[:, c, 128 * t2:128 * t2 + 128],
                                wv_sb[:, c, :],
                                start=False, stop=(c == NC_CHUNKS - 1))
                        nc.vector.memset(vp[tt][:, :, 64], 1.0)
                        nc.vector.tensor_copy(
                            vp[tt][:, :, 0:64],
                            accv[:, 0:JW].rearrange("p (k e) -> p k e", k=HPC))

            # --- phase B: attention
            with contextlib.ExitStack() as esB:
                epool = esB.enter_context(tc.tile_pool(name="epool", bufs=1))
                spool = esB.enter_context(tc.tile_pool(name="spool", bufs=4))
                etile = {}
                for dd in range(2):
                    for tt in range(NT):
                        etile[(dd, tt)] = epool.tile(
                            [128, 1024], BF16, tag=f"e{dd}_{tt}", name=f"e{dd}_{tt}")

                prev_pv_last = None
                for jt in range(2):
                    for fb in range(NFB):
                        fbase = 1024 * fb
                        st_first = None
                        st_last = None
                        pv_first = None
                        pv_last = None
                        E = {}
                        for tt in range(NT):
                            st_pair = []
                            for dd in range(2):
                                stp = psbig.tile([128, 1024], mybir.dt.float32, tag="big", name=f"st{jt}_{fb}_{tt}_{dd}")
                                st_pair.append(stp)
                            for half in range(2):
                                fo = fbase + 512 * half
                                for dd in range(2):
                                    _mm = nc.tensor.matmul(
                                        st_pair[dd][:, 512 * half:512 * half + 512],
                                        kt[jt][64 * dd:64 * dd + 64, 128 * tt:128 * tt + 128],
                                        qt[jt][64 * dd:64 * dd + 64, fo:fo + 512],
                                        start=True, stop=True,
                                        tile_position=(64 * dd, 0))
                                    if st_first is None:
                                        st_first = _mm
                                    st_last = _mm
                            for dd in range(2):
                                e = etile[(dd, tt)]
                                nc.scalar.activation(e[:], st_pair[dd][:],
                                                     Act.Exp, bias=0.0, scale=SCALE)
                                E[(dd, tt)] = e
                        for dd in range(2):
                            k_local = 2 * jt + dd
                            for half in range(2):
                                cacc = pssm.tile([65, 512], mybir.dt.float32, tag="sm")
                                for tt in range(NT):
                                    _pm = nc.tensor.matmul(
                                        cacc[:],
                                        vp[tt][:, k_local, :],
                                        E[(dd, tt)][:, 512 * half:512 * half + 512],
                                        start=(tt == 0), stop=(tt == NT - 1))
                                    if pv_first is None:
                                        pv_first = _pm
                                    pv_last = _pm
                                # epilogue: divide rows 0..63 by row 64
                                rcp = spool.tile([1, 512], BF16, tag="rcp")
                                with nc.allow_low_precision(reason="softmax recip; tol 2e-2"):
                                    nc.vector.reciprocal(rcp[:], cacc[64:65, :])
                                bcp = pssm.tile([64, 512], mybir.dt.float32, tag="sm")
                                nc.tensor.matmul(bcp[:], ones_bf[0:1, :], rcp[:],
                                                 start=True, stop=True)
                                bcs = spool.tile([64, 512], F32, tag="bcs")
                                nc.vector.tensor_copy(bcs[:], bcp[:])
                                so = spool.tile([64, 512], F32, tag="so")
                                nc.vector.tensor_mul(so[:], cacc[0:64, :], bcs[:])
                                nc.sync.dma_start(
                                    out[64 * k_local:64 * k_local + 64,
                                        fbase + 512 * half:fbase + 512 * half + 512],
                                    so[:])
                        # PE-order edges: PV after this round's ST; ST after
                        # previous round's PV (prevents slot-wait deadlocks)
                        add_dep_helper(pv_first.ins, st_last.ins, sync=False,
                                       reason="round PV after round ST on PE")
                        if prev_pv_last is not None:
                            add_dep_helper(st_first.ins, prev_pv_last.ins, sync=False,
                                           reason="round ST after previous round PV")
                        prev_pv_last = pv_last

    nc.compile()
    return nc


def _get_nc():
    global _cached
    if _cached is None:
        _cached = _build()
    return _cached


def _numpy_fallback(from_tensor, to_tensor, attention_mask, Wq, bq, Wk, bk, Wv, bv):
    b, f, _ = from_tensor.shape
    t = to_tensor.shape[1]
    h, d = NUM_HEADS, HEAD_DIM
    q = (from_tensor @ Wq + bq).reshape(b, f, h, d).transpose(0, 2, 1, 3)
    k = (to_tensor @ Wk + bk).reshape(b, t, h, d).transpose(0, 2, 1, 3)
    v = (to_tensor @ Wv + bv).reshape(b, t, h, d).transpose(0, 2, 1, 3)
    scores = np.einsum("bhfd,bhtd->bhft", q, k) * (1.0 / np.sqrt(float(d)))
    adder = (1.0 - attention_mask[:, None].astype(np.float32)) * -10000.0
    scores = scores + adder
    scores = scores - scores.max(axis=-1, keepdims=True)
    e = np.exp(scores)
    probs = e / e.sum(axis=-1, keepdims=True)
    ctx = np.einsum("bhft,bhtd->bhfd", probs, v)
    return ctx.transpose(0, 2, 1, 3).reshape(b, f, h * d).astype(np.float32)


def _make_in_maps(from_tensor, to_tensor, Wq, bq, Wk, bk, Wv, bv):
    fromT = [np.ascontiguousarray(from_tensor[b].T) for b in range(B)]
    toT = [np.ascontiguousarray(to_tensor[b].T) for b in range(B)]
    in_maps = []
    for core in range(8):
        b, g = divmod(core, G)
        j0 = JW * g
        in_maps.append({
            "fromT": fromT[b],
            "toT": toT[b],
            "wq": np.ascontiguousarray(Wq[:, j0:j0 + JW]),
            "wk": np.ascontiguousarray(Wk[:, j0:j0 + JW]),
            "wv": np.ascontiguousarray(Wv[:, j0:j0 + JW]),
            "bq": np.ascontiguousarray(bq[j0:j0 + JW].reshape(JW, 1)),
            "bk": np.ascontiguousarray(bk[j0:j0 + JW].reshape(JW, 1)),
            "bv": np.ascontiguousarray(bv[j0:j0 + JW].reshape(1, JW)),
        })
    return in_maps


def profile_exec_time(inputs):
    """Rerun on HW with NTFF tracing; returns whole-NEFF exec time in ns."""
    from concourse import bass_utils
    nc = _get_nc()
    in_maps = _make_in_maps(
        np.asarray(inputs["from_tensor"], dtype=np.float32),
        np.asarray(inputs["to_tensor"], dtype=np.float32),
        np.asarray(inputs["Wq"], dtype=np.float32),
        np.asarray(inputs["bq"], dtype=np.float32),
        np.asarray(inputs["Wk"], dtype=np.float32),
        np.asarray(inputs["bk"], dtype=np.float32),
        np.asarray(inputs["Wv"], dtype=np.float32),
        np.asarray(inputs["bv"], dtype=np.float32))
    res = bass_utils.run_bass_kernel_spmd(nc, in_maps, core_ids=list(range(8)),
                                          trace=True)
    profile_exec_time.last_results = res
    return res.exec_time_ns


def kernel(**inputs) -> np.ndarray:
    from_tensor = np.asarray(inputs["from_tensor"], dtype=np.float32)
    to_tensor = np.asarray(inputs["to_tensor"], dtype=np.float32)
    attention_mask = np.asarray(inputs["attention_mask"])
    Wq = np.asarray(inputs["Wq"], dtype=np.float32)
    bq = np.asarray(inputs["bq"], dtype=np.float32)
    Wk = np.asarray(inputs["Wk"], dtype=np.float32)
    bk = np.asarray(inputs["bk"], dtype=np.float32)
    Wv = np.asarray(inputs["Wv"], dtype=np.float32)
    bv = np.asarray(inputs["bv"], dtype=np.float32)

    if not np.all(attention_mask == 1):
        # General-mask path (not exercised by the spec'd all-ones fill):
        # plain numpy reference math.
        return _numpy_fallback(from_tensor, to_tensor, attention_mask,
                               Wq, bq, Wk, bk, Wv, bv)

    from concourse import bass_utils

    nc = _get_nc()

    in_maps = _make_in_maps(from_tensor, to_tensor, Wq, bq, Wk, bk, Wv, bv)
    res = bass_utils.run_bass_kernel_spmd(nc, in_maps, core_ids=list(range(8)))
    kernel.last_results = res

    output = np.empty((B, S, HID), dtype=np.float32)
    for core in range(8):
        b, g = divmod(core, G)
        j0 = JW * g
        output[b, :, j0:j0 + JW] = res.results[core]["out"].T
    return output


if __name__ == "__main__":
    rng = np.random.default_rng(0)
    ins = {
        "from_tensor": rng.standard_normal((B, S, HID), dtype=np.float32),
        "to_tensor": rng.standard_normal((B, S, HID), dtype=np.float32),
        "attention_mask": np.ones((B, S, S), dtype=np.int32),
        "Wq": rng.standard_normal((HID, HID), dtype=np.float32) * 0.02,
        "bq": rng.standard_normal((HID,), dtype=np.float32) * 0.01,
        "Wk": rng.standard_normal((HID, HID), dtype=np.float32) * 0.02,
        "bk": rng.standard_normal((HID,), dtype=np.float32) * 0.01,
        "Wv": rng.standard_normal((HID, HID), dtype=np.float32) * 0.02,
        "bv": rng.standard_normal((HID,), dtype=np.float32) * 0.01,
    }
    got = kernel(**ins)
    want = _numpy_fallback(**ins)
    err = np.abs(got - want).max() / np.abs(want).max()
    print("self-test rel err:", err)



# revision 5
# speedup vs baseline: 1.7194x; 1.7194x over previous
"""Trainium2 Bass kernel for nn_AttentionLayer (B=2, S=2048, HID=1024, 16 heads x 64).

Sharding: 8 cores = 2 batches x 4 head-groups (4 heads each). Each core computes
its batch's attention for its 4 heads and writes a disjoint [256, 2048] slice of
the output (transposed). No collectives.

v2 design notes (all-bf16 datapath, ScalarE-exp is the pacing engine):
  - inputs/weights stream in as bf16 (halves HBM traffic vs f32).
  - Q.T/K.T projections accumulate in PSUM f32; DVE adds bias + casts to bf16
    (keeps ScalarE free for exp only).
  - scores: S.T tiles [128t x 1024f] per head via row-packed K=64 matmuls;
    exp on ScalarE -> E bf16. PV matmuls (V'|ones stationary) trail the exp
    stream by one t-tile so the PE never waits on ScalarE.
  - softmax epilogue runs entirely off the PE: DVE copies cacc [65,512] to
    SBUF (frees the PSUM bank), reciprocal_approx_fast on the denominator row,
    GpSimd partition-broadcast, DVE multiply, DMA out.
  - phase A tails (V t-tiles 8-15, K jt1, Q fb1) are interleaved into the
    first rounds' instruction stream to fill PE slack under the exp pacing.
"""
import numpy as np

B, S, HID = 2, 2048, 1024
NUM_HEADS, HEAD_DIM = 16, 64
G = 4                 # head-groups (cores per batch)
HPC = 4               # heads per core
JW = HPC * HEAD_DIM   # 256 W columns per core
NC_CHUNKS = HID // 128  # 8 contraction chunks
NT = S // 128         # 16 t tiles
NFB = 2               # f blocks of 1024
SCALE = 1.0 / np.sqrt(float(HEAD_DIM))

_cached = None


def _build():
    import contextlib
    import concourse.bass as bass
    import concourse.tile as tile
    from concourse import bacc, mybir

    F32 = mybir.dt.float32
    BF16 = mybir.dt.bfloat16
    Act = mybir.ActivationFunctionType

    nc = bacc.Bacc("TRN2", target_bir_lowering=False, debug=False, num_devices=8)

    fromT = nc.dram_tensor("fromT", (HID, S), BF16, kind="ExternalInput").ap()
    toT = nc.dram_tensor("toT", (HID, S), BF16, kind="ExternalInput").ap()
    wq = nc.dram_tensor("wq", (HID, JW), BF16, kind="ExternalInput").ap()
    wk = nc.dram_tensor("wk", (HID, JW), BF16, kind="ExternalInput").ap()
    wv = nc.dram_tensor("wv", (HID, JW), BF16, kind="ExternalInput").ap()
    bq = nc.dram_tensor("bq", (JW, 1), F32, kind="ExternalInput").ap()
    bk = nc.dram_tensor("bk", (JW, 1), F32, kind="ExternalInput").ap()
    bv = nc.dram_tensor("bv", (1, JW), BF16, kind="ExternalInput").ap()
    out = nc.dram_tensor("out", (JW, S), F32, kind="ExternalOutput").ap()

    with tile.TileContext(nc) as tc:
        with contextlib.ExitStack() as es:
            persist = es.enter_context(tc.tile_pool(name="persist", bufs=1))
            psbig = es.enter_context(tc.tile_pool(name="psbig", bufs=2, space="PSUM"))
            pssm = es.enter_context(tc.tile_pool(name="pssm", bufs=4, space="PSUM"))
            work = es.enter_context(tc.tile_pool(name="work", bufs=1))

            # --- constants / biases
            bq_sb = []
            bk_sb = []
            for jt in range(2):
                tq = persist.tile([128, 1], F32, tag=f"bq{jt}")
                nc.sync.dma_start(tq[:], bq[128 * jt:128 * jt + 128, 0:1])
                bq_sb.append(tq)
                tk = persist.tile([128, 1], F32, tag=f"bk{jt}")
                nc.sync.dma_start(tk[:], bk[128 * jt:128 * jt + 128, 0:1])
                bk_sb.append(tk)
            bv_row = persist.tile([1, JW], BF16, tag="bvrow")
            nc.sync.dma_start(bv_row[:], bv[0:1, :])
            ones_bf = persist.tile([1, 128], BF16, tag="onesbf")
            nc.vector.memset(ones_bf[:], 1.0)

            # --- persistent projection outputs (all bf16)
            qt = [persist.tile([128, S], BF16, tag=f"qt{jt}", name=f"qt{jt}") for jt in range(2)]
            kt = [persist.tile([128, S], BF16, tag=f"kt{jt}", name=f"kt{jt}") for jt in range(2)]
            vp = [persist.tile([128, HPC, 65], BF16, tag=f"vp{tt}", name=f"vp{tt}") for tt in range(NT)]

            # --- weights
            wq_sb = persist.tile([128, NC_CHUNKS, JW], BF16, tag="wq")
            nc.sync.dma_start(wq_sb[:], wq.rearrange("(c p) j -> p c j", p=128))
            wk_sb = persist.tile([128, NC_CHUNKS, JW], BF16, tag="wk")
            nc.sync.dma_start(wk_sb[:], wk.rearrange("(c p) j -> p c j", p=128))
            wv_sb = persist.tile([128, NC_CHUNKS, JW], BF16, tag="wv")
            nc.sync.dma_start(wv_sb[:], wv.rearrange("(c p) j -> p c j", p=128))

            # --- input streams: 4 column-halves of 1024 (16KB/partition bf16)
            SH = 1024
            xh = {}
            for (nm, src, h) in (("t0", toT, 0), ("t1", toT, 1),
                                 ("f0", fromT, 0), ("f1", fromT, 1)):
                x = work.tile([128, NC_CHUNKS, SH], BF16, tag="xh",
                              name=f"x_{nm}", bufs=4)
                for c in range(NC_CHUNKS):
                    nc.sync.dma_start(
                        x[:, c, :],
                        src[128 * c:128 * c + 128, SH * h:SH * h + SH])
                xh[nm] = x

            # ---- helper: V projection for one t-tile (9 matmuls + DVE copy)
            def v_proj(tt):
                h, t2 = divmod(tt, NT // 2)
                tx = xh[f"t{h}"]
                # "big" pool, not "sm": the sm slots are all held by a round's
                # cacc tiles while v_proj runs as round-0 filler.
                accv = psbig.tile([128, 512], F32, tag="big", name=f"vacc{tt}")
                nc.tensor.matmul(accv[:, 0:JW], ones_bf[0:1, :], bv_row[0:1, :],
                                 start=True, stop=False)
                for c in range(NC_CHUNKS):
                    nc.tensor.matmul(
                        accv[:, 0:JW],
                        tx[:, c, 128 * t2:128 * t2 + 128],
                        wv_sb[:, c, :],
                        start=False, stop=(c == NC_CHUNKS - 1))
                nc.vector.memset(vp[tt][:, :, 64], 1.0)
                nc.vector.tensor_copy(
                    vp[tt][:, :, 0:64],
                    accv[:, 0:JW].rearrange("p (k e) -> p k e", k=HPC))

            # ---- helper: one 512-wide K-projection block (8 matmuls + DVE add)
            def k_proj_block(jt, tb):  # tb in 0..3: t columns [512*tb, 512*tb+512)
                h, f2 = divmod(tb, 2)
                tx = xh[f"t{h}"]
                acc = psbig.tile([128, 512], F32, tag="big", name=f"kacc{jt}_{tb}")
                for c in range(NC_CHUNKS):
                    nc.tensor.matmul(
                        acc[:],
                        wk_sb[:, c, 128 * jt:128 * jt + 128],
                        tx[:, c, 512 * f2:512 * f2 + 512],
                        start=(c == 0), stop=(c == NC_CHUNKS - 1))
                nc.vector.tensor_scalar_add(
                    kt[jt][:, 512 * tb:512 * tb + 512], acc[:], bk_sb[jt][:])

            # ---- helper: one 512-wide Q-projection block
            def q_proj_block(jt, fbk):  # fbk in 0..3: f columns [512*fbk, ...)
                h, f2 = divmod(fbk, 2)
                fx = xh[f"f{h}"]
                acc = psbig.tile([128, 512], F32, tag="big", name=f"qacc{jt}_{fbk}")
                for c in range(NC_CHUNKS):
                    nc.tensor.matmul(
                        acc[:],
                        wq_sb[:, c, 128 * jt:128 * jt + 128],
                        fx[:, c, 512 * f2:512 * f2 + 512],
                        start=(c == 0), stop=(c == NC_CHUNKS - 1))
                nc.vector.tensor_scalar_add(
                    qt[jt][:, 512 * fbk:512 * fbk + 512], acc[:], bq_sb[jt][:])

            # --- phase A head: V t-tiles 0-7, K jt0, Q (jt0, fb0)
            for tt in range(8):
                v_proj(tt)
            for tb in range(4):
                k_proj_block(0, tb)
            q_proj_block(0, 0)
            q_proj_block(0, 1)

            # --- phase B: 4 rounds, with phase-A tail work interleaved.
            # round order: (fb0,jt0), (fb0,jt1), (fb1,jt0), (fb1,jt1)
            rounds = [(0, 0), (1, 0), (0, 1), (1, 1)]

            # filler[r][tt] = list of thunks to issue inside round r at t-tile tt
            filler = {r: {tt: [] for tt in range(NT)} for r in range(4)}
            # round 0 first half: V t-tiles 8..15 (vp[tt+8] ready before PV needs it)
            for tt in range(8):
                filler[0][tt].append(lambda tt=tt: v_proj(tt + 8))
            # round 0 second half: K jt1 (4 blocks over 8 slots), Q (jt1, fb0)
            for i in range(4):
                filler[0][8 + 2 * i].append(lambda i=i: k_proj_block(1, i))
            filler[0][13].append(lambda: q_proj_block(1, 0))
            filler[0][15].append(lambda: q_proj_block(1, 1))
            # round 1: Q fb1 for both jt
            filler[1][2].append(lambda: q_proj_block(0, 2))
            filler[1][5].append(lambda: q_proj_block(0, 3))
            filler[1][8].append(lambda: q_proj_block(1, 2))
            filler[1][11].append(lambda: q_proj_block(1, 3))

            def epilogue(r, jt, dd, half, cacc):
                k_local = 2 * jt + dd
                fb = rounds[r][1]
                fo = 1024 * fb + 512 * half
                sbf = work.tile([65, 512], F32, tag="sbf", bufs=4,
                                name=f"sbf{r}_{dd}{half}")
                nc.vector.tensor_copy(sbf[:], cacc[:])
                # reciprocal_approx_fast and partition_broadcast both misread
                # sources at a nonzero partition offset on HW (sim is fine) —
                # DMA the denominator row down to partition 0 first.
                den0 = work.tile([1, 512], F32, tag="den0", bufs=4,
                                 name=f"den0{r}_{dd}{half}")
                nc.sync.dma_start(den0[:], sbf[64:65, :])
                rcp = work.tile([1, 512], F32, tag="rcp", bufs=4,
                                name=f"rcp{r}_{dd}{half}")
                nc.vector.reciprocal_approx_fast(rcp[:], den0[:])
                rcpb = work.tile([64, 512], F32, tag="rcpb", bufs=4,
                                 name=f"rcpb{r}_{dd}{half}")
                nc.gpsimd.partition_broadcast(rcpb[:], rcp[:])
                so = work.tile([64, 512], F32, tag="so", bufs=4,
                               name=f"so{r}_{dd}{half}")
                nc.vector.tensor_mul(so[:], sbf[0:64, :], rcpb[:])
                nc.sync.dma_start(
                    out[64 * k_local:64 * k_local + 64, fo:fo + 512], so[:])

            for r, (jt, fb) in enumerate(rounds):
                fbase = 1024 * fb
                cacc = {}
                for dd in range(2):
                    for half in range(2):
                        cacc[(dd, half)] = pssm.tile(
                            [65, 512], F32, tag="sm", name=f"cacc{r}_{dd}{half}")
                E = {}

                def st_exp(tt):
                    for dd in range(2):
                        stp = psbig.tile([128, 1024], F32, tag="big",
                                         name=f"st{r}_{tt}_{dd}")
                        for half in range(2):
                            fo = fbase + 512 * half
                            nc.tensor.matmul(
                                stp[:, 512 * half:512 * half + 512],
                                kt[jt][64 * dd:64 * dd + 64, 128 * tt:128 * tt + 128],
                                qt[jt][64 * dd:64 * dd + 64, fo:fo + 512],
                                start=True, stop=True,
                                tile_position=(64 * dd, 0))
                        e = work.tile([128, 1024], BF16, tag="et", bufs=6,
                                      name=f"e{r}_{tt}_{dd}")
                        nc.scalar.activation(e[:], stp[:], Act.Exp,
                                             bias=0.0, scale=SCALE)
                        E[(dd, tt)] = e

                def pv(tt):
                    for dd in range(2):
                        k_local = 2 * jt + dd
                        for half in range(2):
                            nc.tensor.matmul(
                                cacc[(dd, half)][:],
                                vp[tt][:, k_local, :],
                                E[(dd, tt)][:, 512 * half:512 * half + 512],
                                start=(tt == 0), stop=(tt == NT - 1))

                for tt in range(NT):
                    st_exp(tt)
                    for thunk in filler[r][tt]:
                        thunk()
                    if tt > 0:
                        pv(tt - 1)
                pv(NT - 1)
                for dd in range(2):
                    for half in range(2):
                        epilogue(r, jt, dd, half, cacc[(dd, half)])

    nc.compile()
    return nc


def _get_nc():
    global _cached
    if _cached is None:
        _cached = _build()
    return _cached


def _numpy_fallback(from_tensor, to_tensor, attention_mask, Wq, bq, Wk, bk, Wv, bv):
    b, f, _ = from_tensor.shape
    t = to_tensor.shape[1]
    h, d = NUM_HEADS, HEAD_DIM
    q = (from_tensor @ Wq + bq).reshape(b, f, h, d).transpose(0, 2, 1, 3)
    k = (to_tensor @ Wk + bk).reshape(b, t, h, d).transpose(0, 2, 1, 3)
    v = (to_tensor @ Wv + bv).reshape(b, t, h, d).transpose(0, 2, 1, 3)
    scores = np.einsum("bhfd,bhtd->bhft", q, k) * (1.0 / np.sqrt(float(d)))
    adder = (1.0 - attention_mask[:, None].astype(np.float32)) * -10000.0
    scores = scores + adder
    scores = scores - scores.max(axis=-1, keepdims=True)
    e = np.exp(scores)
    probs = e / e.sum(axis=-1, keepdims=True)
    ctx = np.einsum("bhft,bhtd->bhfd", probs, v)
    return ctx.transpose(0, 2, 1, 3).reshape(b, f, h * d).astype(np.float32)


def _make_in_maps(from_tensor, to_tensor, Wq, bq, Wk, bk, Wv, bv):
    import ml_dtypes
    bf16 = ml_dtypes.bfloat16
    fromT = [np.ascontiguousarray(from_tensor[b].T).astype(bf16) for b in range(B)]
    toT = [np.ascontiguousarray(to_tensor[b].T).astype(bf16) for b in range(B)]
    in_maps = []
    for core in range(8):
        b, g = divmod(core, G)
        j0 = JW * g
        in_maps.append({
            "fromT": fromT[b],
            "toT": toT[b],
            "wq": np.ascontiguousarray(Wq[:, j0:j0 + JW]).astype(bf16),
            "wk": np.ascontiguousarray(Wk[:, j0:j0 + JW]).astype(bf16),
            "wv": np.ascontiguousarray(Wv[:, j0:j0 + JW]).astype(bf16),
            "bq": np.ascontiguousarray(bq[j0:j0 + JW].reshape(JW, 1)),
            "bk": np.ascontiguousarray(bk[j0:j0 + JW].reshape(JW, 1)),
            "bv": np.ascontiguousarray(bv[j0:j0 + JW].reshape(1, JW)).astype(bf16),
        })
    return in_maps


def profile_exec_time(inputs):
    """Rerun on HW with NTFF tracing; returns whole-NEFF exec time in ns."""
    from concourse import bass_utils
    nc = _get_nc()
    in_maps = _make_in_maps(
        np.asarray(inputs["from_tensor"], dtype=np.float32),
        np.asarray(inputs["to_tensor"], dtype=np.float32),
        np.asarray(inputs["Wq"], dtype=np.float32),
        np.asarray(inputs["bq"], dtype=np.float32),
        np.asarray(inputs["Wk"], dtype=np.float32),
        np.asarray(inputs["bk"], dtype=np.float32),
        np.asarray(inputs["Wv"], dtype=np.float32),
        np.asarray(inputs["bv"], dtype=np.float32))
    res = bass_utils.run_bass_kernel_spmd(nc, in_maps, core_ids=list(range(8)),
                                          trace=True)
    profile_exec_time.last_results = res
    return res.exec_time_ns


def kernel(**inputs) -> np.ndarray:
    from_tensor = np.asarray(inputs["from_tensor"], dtype=np.float32)
    to_tensor = np.asarray(inputs["to_tensor"], dtype=np.float32)
    attention_mask = np.asarray(inputs["attention_mask"])
    Wq = np.asarray(inputs["Wq"], dtype=np.float32)
    bq = np.asarray(inputs["bq"], dtype=np.float32)
    Wk = np.asarray(inputs["Wk"], dtype=np.float32)
    bk = np.asarray(inputs["bk"], dtype=np.float32)
    Wv = np.asarray(inputs["Wv"], dtype=np.float32)
    bv = np.asarray(inputs["bv"], dtype=np.float32)

    if not np.all(attention_mask == 1):
        # General-mask path (not exercised by the spec'd all-ones fill):
        # plain numpy reference math.
        return _numpy_fallback(from_tensor, to_tensor, attention_mask,
                               Wq, bq, Wk, bk, Wv, bv)

    from concourse import bass_utils

    nc = _get_nc()

    in_maps = _make_in_maps(from_tensor, to_tensor, Wq, bq, Wk, bk, Wv, bv)
    res = bass_utils.run_bass_kernel_spmd(nc, in_maps, core_ids=list(range(8)))
    kernel.last_results = res

    output = np.empty((B, S, HID), dtype=np.float32)
    for core in range(8):
        b, g = divmod(core, G)
        j0 = JW * g
        output[b, :, j0:j0 + JW] = res.results[core]["out"].T
    return output


if __name__ == "__main__":
    rng = np.random.default_rng(0)
    ins = {
        "from_tensor": rng.standard_normal((B, S, HID), dtype=np.float32),
        "to_tensor": rng.standard_normal((B, S, HID), dtype=np.float32),
        "attention_mask": np.ones((B, S, S), dtype=np.int32),
        "Wq": rng.standard_normal((HID, HID), dtype=np.float32) * 0.02,
        "bq": rng.standard_normal((HID,), dtype=np.float32) * 0.01,
        "Wk": rng.standard_normal((HID, HID), dtype=np.float32) * 0.02,
        "bk": rng.standard_normal((HID,), dtype=np.float32) * 0.01,
        "Wv": rng.standard_normal((HID, HID), dtype=np.float32) * 0.02,
        "bv": rng.standard_normal((HID,), dtype=np.float32) * 0.01,
    }
    got = kernel(**ins)
    want = _numpy_fallback(**ins)
    err = np.abs(got - want).max() / np.abs(want).max()
    print("self-test rel err:", err)
